# revision 24
# baseline (speedup 1.0000x reference)
"""CRCDLoss Trainium2 kernel (8-core SPMD, Bass/Tile).

Strategy: the reference gathers memory rows for every (b, k) pair
(~1.07 GB of HBM traffic). Every use of the gathered rows reduces to
sums over (b, k) of f(exp(S[b, n]/T)) weighted by the multiplicity
counts cnt[b, n] = #{k : idx_all[b, k] == n}, so instead compute the
dense score matrix with matmuls over the n-sharded banks (each bank
read exactly once) and fold the counts in as log-count biases.

Device program per core (n-shard of 12800 padded bank rows):
  - Both banks are packed as the two fp8 DoubleRow "slots" of one
    stationary [128, 2, 128] = [[v_s/T | 0], [0 | v_t/T]], so a single
    DoubleRow matmul per 512-column window yields PSUM rows 0:64 =
    S_s/T and rows 64:128 = S_t/T at 0.5 PE cycles/column.
  - A second DoubleRow matmul injects ln(cnt) (fp8, -88 for cnt=0)
    into the same PSUM via a stacked-identity stationary, so
    exp(PSUM) = cnt * exp(S/T) elementwise.
  - One Exp activation per 4-window group reads PSUM and accumulates
    M1 = sum cnt*e per partition into a per-group column (ScalarE
    accum_out); no VectorE work at all.
Host: embeds f_s/f_t (tiny), builds counts from the index tensors
while sharding, sums the per-core/per-group partials in float64 and
applies the series expansion of the loss. The m=2 series term is
dropped (validated: shifts the loss by ~2e-5 relative).

All normalizer coupling is algebraic (Z = M1*N/(B*(K+1))), so the 8
cores are fully independent: no collectives.
"""

import sys

import numpy as np

try:
    import concourse.bass as bass  # noqa: F401
except ImportError:
    sys.path.insert(0, "/opt/trn_rl_repo")

import concourse.bacc as bacc
import concourse.bass as bass  # noqa: F811
import concourse.mybir as mybir
import concourse.tile as tile
from concourse.bass_utils import run_bass_kernel_spmd

import ml_dtypes

# ---- problem constants (hardcoded; must match the reference) ----
B = 64
D = 128
S_DIM = 1024
T_DIM = 2048
NCE_K = 16384
KP1 = NCE_K + 1          # 16385
N_DATA = 100000
NCE_T = 0.07
EPS = 1e-7
PN = 1.0 / N_DATA
CVAL = NCE_K * PN + EPS  # c = m*Pn + eps

N_CORES = 8
W = 512                  # matmul window along n (one psum bank)
R = N_DATA // N_CORES    # 12500 bank rows per core (no padding)
# columns per exp/accum group: small first group (queued prefetch DMAs
# interleave descriptors, so a small g0 completes its DMA fast and
# compute starts sooner), fat middle groups (keeps the PE well fed — a
# staged ramp of small groups starves the PE at the transition and the
# clock drop is permanent), short last group for a quick readout tail.
GRP_COLS = [512, 2048, 2048, 2048, 2048, 2048, 1536, 212]
N_GRPS = len(GRP_COLS)
NEG_LC = -88.0           # ln-count sentinel for cnt=0 (exp -> 0 in f32)

F32 = mybir.dt.float32
BF16 = mybir.dt.bfloat16
FP8 = mybir.dt.float8e4
NP_FP8 = ml_dtypes.float8_e4m3

TRACE = False            # test.py can flip this for profiling runs
_CACHE = {}


def _build_program():
    nc = bacc.Bacc("TRN2", target_bir_lowering=False, debug=False,
                   num_devices=N_CORES)

    # ---- I/O ----
    # scoreW [128, 2*128]: DoubleRow stationary, slot0=[vs/T | 0],
    # slot1=[0 | vt/T].  j2 [32, 2*128]: stacked-identity inject
    # stationary.  memC [128, 2*R]: per window 512 cols of bank v2
    # (slot0) then 512 of bank v1 (slot1).  lcnt [32, 2*R]: per window
    # ln-counts rows 0:32 (slot0) then 32:64 (slot1).
    scoreW = nc.dram_tensor("scoreW", [D, 2 * D], FP8, kind="ExternalInput")
    j2 = nc.dram_tensor("j2", [32, 2 * D], FP8, kind="ExternalInput")
    memC = nc.dram_tensor("memC", [D, 2 * R], FP8, kind="ExternalInput")
    lcnt = nc.dram_tensor("lcnt", [32, 2 * R], FP8, kind="ExternalInput")
    out_acc = nc.dram_tensor("out_acc", [D, 16], F32, kind="ExternalOutput")

    DR = mybir.MatmulPerfMode.DoubleRow

    with tile.TileContext(nc) as tc:
        with tc.tile_pool(name="persist", bufs=1) as pp, \
             tc.tile_pool(name="grp", bufs=3) as gp, \
             tc.tile_pool(name="psum", bufs=2, space="PSUM") as psp:

            # stationaries ride the scalar HWDGE queue (the scalar engine
            # only needs them by the first real score, ~7us later), keeping
            # the sync queue free to start streaming memC immediately
            sw = pp.tile([D, 2 * D], FP8, tag="sw")
            nc.scalar.dma_start(out=sw[:], in_=scoreW[:])
            jt = pp.tile([32, 2 * D], FP8, tag="jt")
            nc.scalar.dma_start(out=jt[:], in_=j2[:])
            swr = sw[:].rearrange("p (i m) -> p i m", i=2)
            jtr = jt[:].rearrange("p (i m) -> p i m", i=2)

            # exp act-table preload while the DMAs run (memsets go on the
            # otherwise-idle gpsimd engine so they finish early)
            tiny = pp.tile([D, 1], F32, tag="tiny")
            nc.gpsimd.memset(tiny[:], 0.0)
            tiny_o = pp.tile([D, 1], BF16, tag="tiny_o")
            nc.scalar.activation(out=tiny_o[:], in_=tiny[:],
                                 func=mybir.ActivationFunctionType.Exp)

            # PE pstate warm-up bridging into the first real matmul; uses a
            # memset stationary so it needs no DMA and can start immediately
            wz = pp.tile([D, 2 * W], FP8, tag="wz")
            nc.gpsimd.memset(wz[:], 0.0)
            wzW = pp.tile([D, 2 * D], FP8, tag="wzW")
            nc.gpsimd.memset(wzW[:], 0.0)
            wzr = wz[:].rearrange("p (i n) -> p i n", i=2)
            wzWr = wzW[:].rearrange("p (i m) -> p i m", i=2)
            wu_ps = psp.tile([D, W], F32, tag="ps", name="wu_ps",
                             padded_shape=[D, 4 * W])
            # enough warm-up matmuls to (a) ramp the PE clock and (b) keep
            # the PE busy with no gap until the first group's data lands —
            # an idle gap mid-stream drops the PE clock and it stays down
            for _wu in range(14):
                nc.tensor.matmul(out=wu_ps[:], lhsT=wzWr, rhs=wzr,
                                 start=True, stop=True, perf_mode=DR,
                                 skip_group_check=True)

            maccs = pp.tile([D, 16], F32, tag="maccs")
            nc.gpsimd.memset(maccs[:], 0.0)

            # ---- main loop: score + inject matmuls per group ----
            gpos = [0]
            for x in GRP_COLS:
                gpos.append(gpos[-1] + x)
            for g, gw in enumerate(GRP_COLS):
                csl = slice(2 * gpos[g], 2 * gpos[g + 1])
                lc = gp.tile([32, 2 * gw], FP8, tag="lc", name=f"lc{g}",
                             padded_shape=[32, 8 * W])
                if g == 0:
                    # g0's lcnt rides the scalar queue ahead of the big memC
                    # prefetches so the first inject is never starved
                    nc.scalar.dma_start(out=lc[:], in_=lcnt[:, csl])
                else:
                    nc.gpsimd.dma_start(out=lc[:], in_=lcnt[:, csl])
                mc = gp.tile([D, 2 * gw], FP8, tag="mc", name=f"mc{g}",
                             padded_shape=[D, 8 * W])
                mcr = mc[:].rearrange("p (i n) -> p i n", i=2)
                msr = memC[:, csl].rearrange("p (i n) -> p i n", i=2)
                if gw > 2 * W:
                    # split the group's memC fetch so the first windows'
                    # scores can start before the whole group lands
                    h = gw // 2
                    nc.sync.dma_start(out=mcr[:, :, 0:h], in_=msr[:, :, 0:h])
                    nc.sync.dma_start(out=mcr[:, :, h:gw], in_=msr[:, :, h:gw])
                else:
                    nc.sync.dma_start(out=mc[:], in_=memC[:, csl])

                ps = psp.tile([D, gw], F32, tag="ps", name=f"ps{g}",
                              padded_shape=[D, 4 * W])
                lcr = lc[:].rearrange("p (i n) -> p i n", i=2)
                wins = [(w0, min(W, gw - w0)) for w0 in range(0, gw, W)]
                for w0, wn in wins:
                    wsl = slice(w0, w0 + wn)
                    nc.tensor.matmul(out=ps[:, wsl], lhsT=swr,
                                     rhs=mcr[:, :, wsl],
                                     start=True, stop=False, perf_mode=DR,
                                     skip_group_check=True)
                for w0, wn in wins:
                    wsl = slice(w0, w0 + wn)
                    nc.tensor.matmul(out=ps[:, wsl], lhsT=jtr,
                                     rhs=lcr[:, :, wsl],
                                     start=False, stop=True, perf_mode=DR,
                                     skip_group_check=True)

                scr = gp.tile([D, gw], BF16, tag="scr", name=f"scr{g}",
                              padded_shape=[D, 4 * W])
                nc.scalar.activation(out=scr[:], in_=ps[:],
                                     func=mybir.ActivationFunctionType.Exp,
                                     accum_out=maccs[:, g:g + 1])
                if g == N_GRPS - 2:
                    # ship the settled columns early; only the last
                    # group's column rides the final tiny DMA
                    nc.sync.dma_start(out=out_acc[:, 0:N_GRPS - 1],
                                      in_=maccs[:, 0:N_GRPS - 1])

            nc.sync.dma_start(out=out_acc[:, N_GRPS - 1:N_GRPS],
                              in_=maccs[:, N_GRPS - 1:N_GRPS])

    nc.finalize()
    return nc


def _prepare(f_s, f_t, idx, contrast_idx, Ws, bs, Wt, bt,
             memory_v1, memory_v2):
    f_s = np.asarray(f_s, dtype=np.float32)
    f_t = np.asarray(f_t, dtype=np.float32)
    Ws = np.asarray(Ws, dtype=np.float32)
    Wt = np.asarray(Wt, dtype=np.float32)
    bs = np.asarray(bs, dtype=np.float32)
    bt = np.asarray(bt, dtype=np.float32)
    memory_v1 = np.asarray(memory_v1, dtype=np.float32)
    memory_v2 = np.asarray(memory_v2, dtype=np.float32)
    idx = np.asarray(idx).astype(np.int64)
    contrast_idx = np.asarray(contrast_idx).astype(np.int64)

    # ---- embed (host, tiny): v = l2norm(f @ W.T + b) ----
    def embed(f, Wm, b):
        v = (f @ Wm.T + b).astype(np.float64)
        return v / np.sqrt((v * v).sum(1, keepdims=True))

    vs = embed(f_s, Ws, bs)       # [B, D] f64
    vt = embed(f_t, Wt, bt)

    # ---- counts from the integer index tensors (sharding metadata) ----
    idx_all = np.concatenate([idx[:, None], contrast_idx[:, 1:]], axis=1)
    cnt = np.zeros((B, N_DATA), dtype=np.float32)
    np.add.at(cnt, (np.repeat(np.arange(B), KP1), idx_all.ravel()), 1.0)
    lcnt_full = np.where(cnt > 0, np.log(np.maximum(cnt, 1e-30)),
                         np.float32(NEG_LC)).astype(NP_FP8)

    # ---- device constants ----
    vs8 = (vs / NCE_T).astype(np.float32).astype(NP_FP8)   # [B, D]
    vt8 = (vt / NCE_T).astype(np.float32).astype(NP_FP8)
    scoreW = np.zeros((D, 2, D), dtype=NP_FP8)
    scoreW[:, 0, 0:B] = vs8.T
    scoreW[:, 1, B:D] = vt8.T
    scoreW = scoreW.reshape(D, 2 * D)

    j2 = np.zeros((32, 2, D), dtype=NP_FP8)
    for i in range(2):
        for p in range(32):
            j = i * 32 + p
            j2[p, i, j] = 1.0
            j2[p, i, j + B] = 1.0
    j2 = j2.reshape(32, 2 * D)

    # ---- sharded streams ----
    m1p = np.ascontiguousarray(memory_v1.T).astype(NP_FP8)
    m2p = np.ascontiguousarray(memory_v2.T).astype(NP_FP8)

    gpos = [0]
    for x in GRP_COLS:
        gpos.append(gpos[-1] + x)

    in_maps = []
    for c in range(N_CORES):
        sl = slice(c * R, (c + 1) * R)
        m2c, m1c = m2p[:, sl], m1p[:, sl]
        lcc = lcnt_full[:, sl]
        # per group block: [slot0 = gw cols of v2 | slot1 = gw cols of v1]
        mC = np.empty((D, 2 * R), dtype=NP_FP8)
        lC = np.empty((32, 2 * R), dtype=NP_FP8)
        for g, gw in enumerate(GRP_COLS):
            g0, g1 = gpos[g], gpos[g + 1]
            blk = slice(2 * g0, 2 * g0 + gw)
            blk2 = slice(2 * g0 + gw, 2 * g1)
            mC[:, blk] = m2c[:, g0:g1]
            mC[:, blk2] = m1c[:, g0:g1]
            lC[:, blk] = lcc[0:32, g0:g1]
            lC[:, blk2] = lcc[32:64, g0:g1]
        in_maps.append({"scoreW": scoreW, "j2": j2, "memC": mC, "lcnt": lC})

    possum = (
        (memory_v2[idx].astype(np.float64) * vs).sum() / NCE_T,
        (memory_v1[idx].astype(np.float64) * vt).sum() / NCE_T,
    )
    return in_maps, possum


def _combine(out_accs, possum):
    """out_accs: per-core [128, 16] f32 -> scalar loss (float32)."""
    outs = np.stack([np.asarray(o).astype(np.float64) for o in out_accs])

    loss = 0.0
    for side, half in enumerate((slice(0, B), slice(B, D))):
        M1 = outs[:, half, :].sum()
        Z = M1 / (B * KP1) * N_DATA
        cz = CVAL * Z
        sum_ln_xc = B * KP1 * np.log(CVAL) + M1 / cz
        neg_b = (possum[side] - B * np.log(Z)
                 + B * NCE_K * np.log(NCE_K * PN) - sum_ln_xc)
        loss += -neg_b / B
    return np.float32(loss)


def kernel(f_s, f_t, idx, contrast_idx, Ws, bs, Wt, bt, memory_v1, memory_v2):
    in_maps, possum = _prepare(f_s, f_t, idx, contrast_idx, Ws, bs, Wt, bt,
                               memory_v1, memory_v2)
    if "nc" not in _CACHE:
        _CACHE["nc"] = _build_program()
    nc = _CACHE["nc"]
    res = run_bass_kernel_spmd(nc, in_maps, list(range(N_CORES)), trace=TRACE)
    _CACHE["last_results"] = res
    _CACHE["last_possum"] = possum
    return kernel_combine_results(res)


def kernel_combine_results(res):
    return _combine([res.results[c]["out_acc"] for c in range(N_CORES)],
                    _CACHE["last_possum"])


# revision 25
# speedup vs baseline: 1.1617x; 1.1617x over previous
"""CRCDLoss Trainium2 kernel (8-core SPMD, Bass/Tile).

Strategy: the reference gathers memory rows for every (b, k) pair
(~1.07 GB of HBM traffic). Every use of the gathered rows reduces to
sums over (b, k) of f(exp(S[b, n]/T)) weighted by the multiplicity
counts cnt[b, n] = #{k : idx_all[b, k] == n}, so instead compute the
dense score matrix with matmuls over the n-sharded banks (each bank
read exactly once) and fold the counts in as log-count biases.

Device program per core (n-shard of 12800 padded bank rows):
  - Both banks are packed as the two fp8 DoubleRow "slots" of one
    stationary [128, 2, 128] = [[v_s/T | 0], [0 | v_t/T]], so a single
    DoubleRow matmul per 512-column window yields PSUM rows 0:64 =
    S_s/T and rows 64:128 = S_t/T at 0.5 PE cycles/column.
  - A second DoubleRow matmul injects ln(cnt) (fp8, -88 for cnt=0)
    into the same PSUM via a stacked-identity stationary, so
    exp(PSUM) = cnt * exp(S/T) elementwise.
  - One Exp activation per 4-window group reads PSUM and accumulates
    M1 = sum cnt*e per partition into a per-group column (ScalarE
    accum_out); no VectorE work at all.
Host: embeds f_s/f_t (tiny), builds counts from the index tensors
while sharding, sums the per-core/per-group partials in float64 and
applies the series expansion of the loss. The m=2 series term is
dropped (validated: shifts the loss by ~2e-5 relative).

All normalizer coupling is algebraic (Z = M1*N/(B*(K+1))), so the 8
cores are fully independent: no collectives.
"""

import sys

import numpy as np

try:
    import concourse.bass as bass  # noqa: F401
except ImportError:
    sys.path.insert(0, "/opt/trn_rl_repo")

import concourse.bacc as bacc
import concourse.bass as bass  # noqa: F811
import concourse.mybir as mybir
import concourse.tile as tile
from concourse.bass_utils import run_bass_kernel_spmd

import ml_dtypes

# ---- problem constants (hardcoded; must match the reference) ----
B = 64
D = 128
S_DIM = 1024
T_DIM = 2048
NCE_K = 16384
KP1 = NCE_K + 1          # 16385
N_DATA = 100000
NCE_T = 0.07
EPS = 1e-7
PN = 1.0 / N_DATA
CVAL = NCE_K * PN + EPS  # c = m*Pn + eps

N_CORES = 8
W = 512                  # matmul window along n (one psum bank)
R = N_DATA // N_CORES    # 12500 bank rows per core (no padding)
# columns per exp/accum group: small first group (queued prefetch DMAs
# interleave descriptors, so a small g0 completes its DMA fast and
# compute starts sooner), fat middle groups (keeps the PE well fed — a
# staged ramp of small groups starves the PE at the transition and the
# clock drop is permanent), short last group for a quick readout tail.
GRP_COLS = [512, 2048, 2048, 2048, 2048, 2048, 1536, 212]
N_GRPS = len(GRP_COLS)
NEG_LC = -88.0           # ln-count sentinel for cnt=0 (exp -> 0 in f32)

F32 = mybir.dt.float32
BF16 = mybir.dt.bfloat16
FP8 = mybir.dt.float8e4
NP_FP8 = ml_dtypes.float8_e4m3

TRACE = False            # test.py can flip this for profiling runs
_CACHE = {}


def _build_program():
    nc = bacc.Bacc("TRN2", target_bir_lowering=False, debug=False,
                   num_devices=N_CORES)

    # ---- I/O ----
    # scoreW [128, 2*128]: DoubleRow stationary, slot0=[vs/T | 0],
    # slot1=[0 | vt/T].  j2 [32, 2*128]: stacked-identity inject
    # stationary.  memC [128, 2*R]: per window 512 cols of bank v2
    # (slot0) then 512 of bank v1 (slot1).  lcnt [32, 2*R]: per window
    # ln-counts rows 0:32 (slot0) then 32:64 (slot1).
    scoreW = nc.dram_tensor("scoreW", [D, 2 * D], FP8, kind="ExternalInput")
    j2 = nc.dram_tensor("j2", [32, 2 * D], FP8, kind="ExternalInput")
    memC = nc.dram_tensor("memC", [D, 2 * R], FP8, kind="ExternalInput")
    lcnt = nc.dram_tensor("lcnt", [32, 2 * R], FP8, kind="ExternalInput")
    out_acc = nc.dram_tensor("out_acc", [D, 16], F32, kind="ExternalOutput")

    DR = mybir.MatmulPerfMode.DoubleRow

    with tile.TileContext(nc) as tc:
        with tc.tile_pool(name="persist", bufs=1) as pp, \
             tc.tile_pool(name="grp", bufs=3) as gp, \
             tc.tile_pool(name="psum", bufs=2, space="PSUM") as psp:

            # stationaries ride the scalar HWDGE queue (the scalar engine
            # only needs them by the first real score, ~7us later), keeping
            # the sync queue free to start streaming memC immediately
            sw = pp.tile([D, 2 * D], FP8, tag="sw")
            nc.scalar.dma_start(out=sw[:], in_=scoreW[:])
            jt = pp.tile([32, 2 * D], FP8, tag="jt")
            nc.scalar.dma_start(out=jt[:], in_=j2[:])
            swr = sw[:].rearrange("p (i m) -> p i m", i=2)
            jtr = jt[:].rearrange("p (i m) -> p i m", i=2)

            # exp act-table preload while the DMAs run (memsets go on the
            # otherwise-idle gpsimd engine so they finish early)
            tiny = pp.tile([D, 1], F32, tag="tiny")
            nc.gpsimd.memset(tiny[:], 0.0)
            tiny_o = pp.tile([D, 1], BF16, tag="tiny_o")
            nc.scalar.activation(out=tiny_o[:], in_=tiny[:],
                                 func=mybir.ActivationFunctionType.Exp)

            # PE pstate warm-up bridging into the first real matmul; uses a
            # memset stationary so it needs no DMA and can start immediately
            wz = pp.tile([D, 2 * W], FP8, tag="wz")
            nc.gpsimd.memset(wz[:], 0.0)
            wzW = pp.tile([D, 2 * D], FP8, tag="wzW")
            nc.gpsimd.memset(wzW[:], 0.0)
            wzr = wz[:].rearrange("p (i n) -> p i n", i=2)
            wzWr = wzW[:].rearrange("p (i m) -> p i m", i=2)
            wu_ps = psp.tile([D, W], F32, tag="ps", name="wu_ps",
                             padded_shape=[D, 4 * W])
            # enough warm-up matmuls to (a) ramp the PE clock and (b) keep
            # the PE busy with no gap until the first group's data lands —
            # an idle gap mid-stream drops the PE clock and it stays down
            for _wu in range(14):
                nc.tensor.matmul(out=wu_ps[:], lhsT=wzWr, rhs=wzr,
                                 start=True, stop=True, perf_mode=DR,
                                 skip_group_check=True)

            maccs = pp.tile([D, 16], F32, tag="maccs")
            nc.gpsimd.memset(maccs[:], 0.0)

            # ---- main loop: score + inject matmuls per group ----
            gpos = [0]
            for x in GRP_COLS:
                gpos.append(gpos[-1] + x)
            for g, gw in enumerate(GRP_COLS):
                csl = slice(2 * gpos[g], 2 * gpos[g + 1])
                lc = gp.tile([32, 2 * gw], FP8, tag="lc", name=f"lc{g}",
                             padded_shape=[32, 8 * W])
                if g == 0:
                    # g0's lcnt rides the sync queue ahead of the big memC
                    # prefetches so the first inject is never starved
                    nc.sync.dma_start(out=lc[:], in_=lcnt[:, csl])
                else:
                    nc.gpsimd.dma_start(out=lc[:], in_=lcnt[:, csl])
                mc = gp.tile([D, 2 * gw], FP8, tag="mc", name=f"mc{g}",
                             padded_shape=[D, 8 * W])
                mcr = mc[:].rearrange("p (i n) -> p i n", i=2)
                msr = memC[:, csl].rearrange("p (i n) -> p i n", i=2)
                if gw > 2 * W:
                    # split the group's memC fetch so the first windows'
                    # scores can start before the whole group lands
                    h = gw // 2
                    nc.sync.dma_start(out=mcr[:, :, 0:h], in_=msr[:, :, 0:h])
                    nc.sync.dma_start(out=mcr[:, :, h:gw], in_=msr[:, :, h:gw])
                else:
                    nc.sync.dma_start(out=mc[:], in_=memC[:, csl])

                ps = psp.tile([D, gw], F32, tag="ps", name=f"ps{g}",
                              padded_shape=[D, 4 * W])
                lcr = lc[:].rearrange("p (i n) -> p i n", i=2)
                wins = [(w0, min(W, gw - w0)) for w0 in range(0, gw, W)]
                for w0, wn in wins:
                    wsl = slice(w0, w0 + wn)
                    nc.tensor.matmul(out=ps[:, wsl], lhsT=swr,
                                     rhs=mcr[:, :, wsl],
                                     start=True, stop=False, perf_mode=DR,
                                     skip_group_check=True)
                for w0, wn in wins:
                    wsl = slice(w0, w0 + wn)
                    nc.tensor.matmul(out=ps[:, wsl], lhsT=jtr,
                                     rhs=lcr[:, :, wsl],
                                     start=False, stop=True, perf_mode=DR,
                                     skip_group_check=True)

                scr = gp.tile([D, gw], BF16, tag="scr", name=f"scr{g}",
                              padded_shape=[D, 4 * W])
                nc.scalar.activation(out=scr[:], in_=ps[:],
                                     func=mybir.ActivationFunctionType.Exp,
                                     accum_out=maccs[:, g:g + 1])
                if g == N_GRPS - 2:
                    # ship the settled columns early; only the last
                    # group's column rides the final tiny DMA
                    nc.sync.dma_start(out=out_acc[:, 0:N_GRPS - 1],
                                      in_=maccs[:, 0:N_GRPS - 1])

            nc.sync.dma_start(out=out_acc[:, N_GRPS - 1:N_GRPS],
                              in_=maccs[:, N_GRPS - 1:N_GRPS])

    nc.finalize()
    return nc


def _prepare(f_s, f_t, idx, contrast_idx, Ws, bs, Wt, bt,
             memory_v1, memory_v2):
    f_s = np.asarray(f_s, dtype=np.float32)
    f_t = np.asarray(f_t, dtype=np.float32)
    Ws = np.asarray(Ws, dtype=np.float32)
    Wt = np.asarray(Wt, dtype=np.float32)
    bs = np.asarray(bs, dtype=np.float32)
    bt = np.asarray(bt, dtype=np.float32)
    memory_v1 = np.asarray(memory_v1, dtype=np.float32)
    memory_v2 = np.asarray(memory_v2, dtype=np.float32)
    idx = np.asarray(idx).astype(np.int64)
    contrast_idx = np.asarray(contrast_idx).astype(np.int64)

    # ---- embed (host, tiny): v = l2norm(f @ W.T + b) ----
    def embed(f, Wm, b):
        v = (f @ Wm.T + b).astype(np.float64)
        return v / np.sqrt((v * v).sum(1, keepdims=True))

    vs = embed(f_s, Ws, bs)       # [B, D] f64
    vt = embed(f_t, Wt, bt)

    # ---- counts from the integer index tensors (sharding metadata) ----
    idx_all = np.concatenate([idx[:, None], contrast_idx[:, 1:]], axis=1)
    cnt = np.zeros((B, N_DATA), dtype=np.float32)
    np.add.at(cnt, (np.repeat(np.arange(B), KP1), idx_all.ravel()), 1.0)
    lcnt_full = np.where(cnt > 0, np.log(np.maximum(cnt, 1e-30)),
                         np.float32(NEG_LC)).astype(NP_FP8)

    # ---- device constants ----
    vs8 = (vs / NCE_T).astype(np.float32).astype(NP_FP8)   # [B, D]
    vt8 = (vt / NCE_T).astype(np.float32).astype(NP_FP8)
    scoreW = np.zeros((D, 2, D), dtype=NP_FP8)
    scoreW[:, 0, 0:B] = vs8.T
    scoreW[:, 1, B:D] = vt8.T
    scoreW = scoreW.reshape(D, 2 * D)

    j2 = np.zeros((32, 2, D), dtype=NP_FP8)
    for i in range(2):
        for p in range(32):
            j = i * 32 + p
            j2[p, i, j] = 1.0
            j2[p, i, j + B] = 1.0
    j2 = j2.reshape(32, 2 * D)

    # ---- sharded streams ----
    m1p = np.ascontiguousarray(memory_v1.T).astype(NP_FP8)
    m2p = np.ascontiguousarray(memory_v2.T).astype(NP_FP8)

    gpos = [0]
    for x in GRP_COLS:
        gpos.append(gpos[-1] + x)

    in_maps = []
    for c in range(N_CORES):
        sl = slice(c * R, (c + 1) * R)
        m2c, m1c = m2p[:, sl], m1p[:, sl]
        lcc = lcnt_full[:, sl]
        # per group block: [slot0 = gw cols of v2 | slot1 = gw cols of v1]
        mC = np.empty((D, 2 * R), dtype=NP_FP8)
        lC = np.empty((32, 2 * R), dtype=NP_FP8)
        for g, gw in enumerate(GRP_COLS):
            g0, g1 = gpos[g], gpos[g + 1]
            blk = slice(2 * g0, 2 * g0 + gw)
            blk2 = slice(2 * g0 + gw, 2 * g1)
            mC[:, blk] = m2c[:, g0:g1]
            mC[:, blk2] = m1c[:, g0:g1]
            lC[:, blk] = lcc[0:32, g0:g1]
            lC[:, blk2] = lcc[32:64, g0:g1]
        in_maps.append({"scoreW": scoreW, "j2": j2, "memC": mC, "lcnt": lC})

    possum = (
        (memory_v2[idx].astype(np.float64) * vs).sum() / NCE_T,
        (memory_v1[idx].astype(np.float64) * vt).sum() / NCE_T,
    )
    return in_maps, possum


def _combine(out_accs, possum):
    """out_accs: per-core [128, 16] f32 -> scalar loss (float32)."""
    outs = np.stack([np.asarray(o).astype(np.float64) for o in out_accs])

    loss = 0.0
    for side, half in enumerate((slice(0, B), slice(B, D))):
        M1 = outs[:, half, :].sum()
        Z = M1 / (B * KP1) * N_DATA
        cz = CVAL * Z
        sum_ln_xc = B * KP1 * np.log(CVAL) + M1 / cz
        neg_b = (possum[side] - B * np.log(Z)
                 + B * NCE_K * np.log(NCE_K * PN) - sum_ln_xc)
        loss += -neg_b / B
    return np.float32(loss)


def kernel(f_s, f_t, idx, contrast_idx, Ws, bs, Wt, bt, memory_v1, memory_v2):
    in_maps, possum = _prepare(f_s, f_t, idx, contrast_idx, Ws, bs, Wt, bt,
                               memory_v1, memory_v2)
    if "nc" not in _CACHE:
        _CACHE["nc"] = _build_program()
    nc = _CACHE["nc"]
    res = run_bass_kernel_spmd(nc, in_maps, list(range(N_CORES)), trace=TRACE)
    _CACHE["last_results"] = res
    _CACHE["last_possum"] = possum
    return kernel_combine_results(res)


def kernel_combine_results(res):
    return _combine([res.results[c]["out_acc"] for c in range(N_CORES)],
                    _CACHE["last_possum"])


# revision 26
# speedup vs baseline: 1.1984x; 1.0316x over previous
"""CRCDLoss Trainium2 kernel (8-core SPMD, Bass/Tile).

Strategy: the reference gathers memory rows for every (b, k) pair
(~1.07 GB of HBM traffic). Every use of the gathered rows reduces to
sums over (b, k) of f(exp(S[b, n]/T)) weighted by the multiplicity
counts cnt[b, n] = #{k : idx_all[b, k] == n}, so instead compute the
dense score matrix with matmuls over the n-sharded banks (each bank
read exactly once) and fold the counts in as log-count biases.

Device program per core (n-shard of 12800 padded bank rows):
  - Both banks are packed as the two fp8 DoubleRow "slots" of one
    stationary [128, 2, 128] = [[v_s/T | 0], [0 | v_t/T]], so a single
    DoubleRow matmul per 512-column window yields PSUM rows 0:64 =
    S_s/T and rows 64:128 = S_t/T at 0.5 PE cycles/column.
  - A second DoubleRow matmul injects ln(cnt) (fp8, -88 for cnt=0)
    into the same PSUM via a stacked-identity stationary, so
    exp(PSUM) = cnt * exp(S/T) elementwise.
  - One Exp activation per 4-window group reads PSUM and accumulates
    M1 = sum cnt*e per partition into a per-group column (ScalarE
    accum_out); no VectorE work at all.
Host: embeds f_s/f_t (tiny), builds counts from the index tensors
while sharding, sums the per-core/per-group partials in float64 and
applies the series expansion of the loss. The m=2 series term is
dropped (validated: shifts the loss by ~2e-5 relative).

All normalizer coupling is algebraic (Z = M1*N/(B*(K+1))), so the 8
cores are fully independent: no collectives.
"""

import sys

import numpy as np

try:
    import concourse.bass as bass  # noqa: F401
except ImportError:
    sys.path.insert(0, "/opt/trn_rl_repo")

import concourse.bacc as bacc
import concourse.bass as bass  # noqa: F811
import concourse.mybir as mybir
import concourse.tile as tile
from concourse.bass_utils import run_bass_kernel_spmd

import ml_dtypes

# ---- problem constants (hardcoded; must match the reference) ----
B = 64
D = 128
S_DIM = 1024
T_DIM = 2048
NCE_K = 16384
KP1 = NCE_K + 1          # 16385
N_DATA = 100000
NCE_T = 0.07
EPS = 1e-7
PN = 1.0 / N_DATA
CVAL = NCE_K * PN + EPS  # c = m*Pn + eps

N_CORES = 8
W = 512                  # matmul window along n (one psum bank)
R = N_DATA // N_CORES    # 12500 bank rows per core (no padding)
# columns per exp/accum group: small first group (queued prefetch DMAs
# interleave descriptors, so a small g0 completes its DMA fast and
# compute starts sooner), fat middle groups (keeps the PE well fed — a
# staged ramp of small groups starves the PE at the transition and the
# clock drop is permanent), short last group for a quick readout tail.
GRP_COLS = [512, 1536, 2048, 2048, 2048, 2048, 2048, 212]
N_GRPS = len(GRP_COLS)
NEG_LC = -88.0           # ln-count sentinel for cnt=0 (exp -> 0 in f32)

F32 = mybir.dt.float32
BF16 = mybir.dt.bfloat16
FP8 = mybir.dt.float8e4
NP_FP8 = ml_dtypes.float8_e4m3

TRACE = False            # test.py can flip this for profiling runs
_CACHE = {}


def _build_program():
    nc = bacc.Bacc("TRN2", target_bir_lowering=False, debug=False,
                   num_devices=N_CORES)

    # ---- I/O ----
    # scoreW [128, 2*128]: DoubleRow stationary, slot0=[vs/T | 0],
    # slot1=[0 | vt/T].  j2 [32, 2*128]: stacked-identity inject
    # stationary.  memC [128, 2*R]: per window 512 cols of bank v2
    # (slot0) then 512 of bank v1 (slot1).  lcnt [32, 2*R]: per window
    # ln-counts rows 0:32 (slot0) then 32:64 (slot1).
    scoreW = nc.dram_tensor("scoreW", [D, 2 * D], FP8, kind="ExternalInput")
    j2 = nc.dram_tensor("j2", [32, 2 * D], FP8, kind="ExternalInput")
    memC = nc.dram_tensor("memC", [D, 2 * R], FP8, kind="ExternalInput")
    lcnt = nc.dram_tensor("lcnt", [32, 2 * R], FP8, kind="ExternalInput")
    out_acc = nc.dram_tensor("out_acc", [D, 16], F32, kind="ExternalOutput")

    DR = mybir.MatmulPerfMode.DoubleRow

    with tile.TileContext(nc) as tc:
        with tc.tile_pool(name="persist", bufs=1) as pp, \
             tc.tile_pool(name="grp", bufs=3) as gp, \
             tc.tile_pool(name="psum", bufs=2, space="PSUM") as psp:

            # stationaries ride the scalar HWDGE queue (the scalar engine
            # only needs them by the first real score, ~7us later), keeping
            # the sync queue free to start streaming memC immediately
            sw = pp.tile([D, 2 * D], FP8, tag="sw")
            nc.scalar.dma_start(out=sw[:], in_=scoreW[:])
            jt = pp.tile([32, 2 * D], FP8, tag="jt")
            nc.scalar.dma_start(out=jt[:], in_=j2[:])
            swr = sw[:].rearrange("p (i m) -> p i m", i=2)
            jtr = jt[:].rearrange("p (i m) -> p i m", i=2)

            # exp act-table preload while the DMAs run (memsets go on the
            # otherwise-idle gpsimd engine so they finish early)
            tiny = pp.tile([D, 1], F32, tag="tiny")
            nc.gpsimd.memset(tiny[:], 0.0)
            tiny_o = pp.tile([D, 1], BF16, tag="tiny_o")
            nc.scalar.activation(out=tiny_o[:], in_=tiny[:],
                                 func=mybir.ActivationFunctionType.Exp)

            # PE pstate warm-up bridging into the first real matmul; uses a
            # memset stationary so it needs no DMA and can start immediately
            wz = pp.tile([D, 2 * W], FP8, tag="wz")
            nc.gpsimd.memset(wz[:], 0.0)
            wzW = pp.tile([D, 2 * D], FP8, tag="wzW")
            nc.gpsimd.memset(wzW[:], 0.0)
            wzr = wz[:].rearrange("p (i n) -> p i n", i=2)
            wzWr = wzW[:].rearrange("p (i m) -> p i m", i=2)
            wu_ps = psp.tile([D, W], F32, tag="ps", name="wu_ps",
                             padded_shape=[D, 4 * W])
            # enough warm-up matmuls to (a) ramp the PE clock and (b) keep
            # the PE busy with no gap until the first group's data lands —
            # an idle gap mid-stream drops the PE clock and it stays down
            for _wu in range(14):
                nc.tensor.matmul(out=wu_ps[:], lhsT=wzWr, rhs=wzr,
                                 start=True, stop=True, perf_mode=DR,
                                 skip_group_check=True)

            maccs = pp.tile([D, 16], F32, tag="maccs")
            nc.gpsimd.memset(maccs[:], 0.0)

            # ---- main loop: score + inject matmuls per group ----
            gpos = [0]
            for x in GRP_COLS:
                gpos.append(gpos[-1] + x)
            for g, gw in enumerate(GRP_COLS):
                csl = slice(2 * gpos[g], 2 * gpos[g + 1])
                lc = gp.tile([32, 2 * gw], FP8, tag="lc", name=f"lc{g}",
                             padded_shape=[32, 8 * W])
                if g == 0:
                    # g0's lcnt rides the sync queue ahead of the big memC
                    # prefetches so the first inject is never starved
                    nc.sync.dma_start(out=lc[:], in_=lcnt[:, csl])
                else:
                    nc.gpsimd.dma_start(out=lc[:], in_=lcnt[:, csl])
                mc = gp.tile([D, 2 * gw], FP8, tag="mc", name=f"mc{g}",
                             padded_shape=[D, 8 * W])
                mcr = mc[:].rearrange("p (i n) -> p i n", i=2)
                msr = memC[:, csl].rearrange("p (i n) -> p i n", i=2)
                if gw > 2 * W:
                    # split the group's memC fetch so the first windows'
                    # scores can start before the whole group lands
                    h = gw // 2
                    nc.sync.dma_start(out=mcr[:, :, 0:h], in_=msr[:, :, 0:h])
                    nc.sync.dma_start(out=mcr[:, :, h:gw], in_=msr[:, :, h:gw])
                else:
                    nc.sync.dma_start(out=mc[:], in_=memC[:, csl])

                ps = psp.tile([D, gw], F32, tag="ps", name=f"ps{g}",
                              padded_shape=[D, 4 * W])
                lcr = lc[:].rearrange("p (i n) -> p i n", i=2)
                wins = [(w0, min(W, gw - w0)) for w0 in range(0, gw, W)]
                for w0, wn in wins:
                    wsl = slice(w0, w0 + wn)
                    nc.tensor.matmul(out=ps[:, wsl], lhsT=swr,
                                     rhs=mcr[:, :, wsl],
                                     start=True, stop=False, perf_mode=DR,
                                     skip_group_check=True)
                for w0, wn in wins:
                    wsl = slice(w0, w0 + wn)
                    nc.tensor.matmul(out=ps[:, wsl], lhsT=jtr,
                                     rhs=lcr[:, :, wsl],
                                     start=False, stop=True, perf_mode=DR,
                                     skip_group_check=True)

                scr = gp.tile([D, gw], BF16, tag="scr", name=f"scr{g}",
                              padded_shape=[D, 4 * W])
                nc.scalar.activation(out=scr[:], in_=ps[:],
                                     func=mybir.ActivationFunctionType.Exp,
                                     accum_out=maccs[:, g:g + 1])
                if g == N_GRPS - 2:
                    # ship the settled columns early; only the last
                    # group's column rides the final tiny DMA
                    nc.sync.dma_start(out=out_acc[:, 0:N_GRPS - 1],
                                      in_=maccs[:, 0:N_GRPS - 1])

            nc.sync.dma_start(out=out_acc[:, N_GRPS - 1:N_GRPS],
                              in_=maccs[:, N_GRPS - 1:N_GRPS])

    nc.finalize()
    return nc


def _prepare(f_s, f_t, idx, contrast_idx, Ws, bs, Wt, bt,
             memory_v1, memory_v2):
    f_s = np.asarray(f_s, dtype=np.float32)
    f_t = np.asarray(f_t, dtype=np.float32)
    Ws = np.asarray(Ws, dtype=np.float32)
    Wt = np.asarray(Wt, dtype=np.float32)
    bs = np.asarray(bs, dtype=np.float32)
    bt = np.asarray(bt, dtype=np.float32)
    memory_v1 = np.asarray(memory_v1, dtype=np.float32)
    memory_v2 = np.asarray(memory_v2, dtype=np.float32)
    idx = np.asarray(idx).astype(np.int64)
    contrast_idx = np.asarray(contrast_idx).astype(np.int64)

    # ---- embed (host, tiny): v = l2norm(f @ W.T + b) ----
    def embed(f, Wm, b):
        v = (f @ Wm.T + b).astype(np.float64)
        return v / np.sqrt((v * v).sum(1, keepdims=True))

    vs = embed(f_s, Ws, bs)       # [B, D] f64
    vt = embed(f_t, Wt, bt)

    # ---- counts from the integer index tensors (sharding metadata) ----
    idx_all = np.concatenate([idx[:, None], contrast_idx[:, 1:]], axis=1)
    cnt = np.zeros((B, N_DATA), dtype=np.float32)
    np.add.at(cnt, (np.repeat(np.arange(B), KP1), idx_all.ravel()), 1.0)
    lcnt_full = np.where(cnt > 0, np.log(np.maximum(cnt, 1e-30)),
                         np.float32(NEG_LC)).astype(NP_FP8)

    # ---- device constants ----
    vs8 = (vs / NCE_T).astype(np.float32).astype(NP_FP8)   # [B, D]
    vt8 = (vt / NCE_T).astype(np.float32).astype(NP_FP8)
    scoreW = np.zeros((D, 2, D), dtype=NP_FP8)
    scoreW[:, 0, 0:B] = vs8.T
    scoreW[:, 1, B:D] = vt8.T
    scoreW = scoreW.reshape(D, 2 * D)

    j2 = np.zeros((32, 2, D), dtype=NP_FP8)
    for i in range(2):
        for p in range(32):
            j = i * 32 + p
            j2[p, i, j] = 1.0
            j2[p, i, j + B] = 1.0
    j2 = j2.reshape(32, 2 * D)

    # ---- sharded streams ----
    m1p = np.ascontiguousarray(memory_v1.T).astype(NP_FP8)
    m2p = np.ascontiguousarray(memory_v2.T).astype(NP_FP8)

    gpos = [0]
    for x in GRP_COLS:
        gpos.append(gpos[-1] + x)

    in_maps = []
    for c in range(N_CORES):
        sl = slice(c * R, (c + 1) * R)
        m2c, m1c = m2p[:, sl], m1p[:, sl]
        lcc = lcnt_full[:, sl]
        # per group block: [slot0 = gw cols of v2 | slot1 = gw cols of v1]
        mC = np.empty((D, 2 * R), dtype=NP_FP8)
        lC = np.empty((32, 2 * R), dtype=NP_FP8)
        for g, gw in enumerate(GRP_COLS):
            g0, g1 = gpos[g], gpos[g + 1]
            blk = slice(2 * g0, 2 * g0 + gw)
            blk2 = slice(2 * g0 + gw, 2 * g1)
            mC[:, blk] = m2c[:, g0:g1]
            mC[:, blk2] = m1c[:, g0:g1]
            lC[:, blk] = lcc[0:32, g0:g1]
            lC[:, blk2] = lcc[32:64, g0:g1]
        in_maps.append({"scoreW": scoreW, "j2": j2, "memC": mC, "lcnt": lC})

    possum = (
        (memory_v2[idx].astype(np.float64) * vs).sum() / NCE_T,
        (memory_v1[idx].astype(np.float64) * vt).sum() / NCE_T,
    )
    return in_maps, possum


def _combine(out_accs, possum):
    """out_accs: per-core [128, 16] f32 -> scalar loss (float32)."""
    outs = np.stack([np.asarray(o).astype(np.float64) for o in out_accs])

    loss = 0.0
    for side, half in enumerate((slice(0, B), slice(B, D))):
        M1 = outs[:, half, :].sum()
        Z = M1 / (B * KP1) * N_DATA
        cz = CVAL * Z
        sum_ln_xc = B * KP1 * np.log(CVAL) + M1 / cz
        neg_b = (possum[side] - B * np.log(Z)
                 + B * NCE_K * np.log(NCE_K * PN) - sum_ln_xc)
        loss += -neg_b / B
    return np.float32(loss)


def kernel(f_s, f_t, idx, contrast_idx, Ws, bs, Wt, bt, memory_v1, memory_v2):
    in_maps, possum = _prepare(f_s, f_t, idx, contrast_idx, Ws, bs, Wt, bt,
                               memory_v1, memory_v2)
    if "nc" not in _CACHE:
        _CACHE["nc"] = _build_program()
    nc = _CACHE["nc"]
    res = run_bass_kernel_spmd(nc, in_maps, list(range(N_CORES)), trace=TRACE)
    _CACHE["last_results"] = res
    _CACHE["last_possum"] = possum
    return kernel_combine_results(res)


def kernel_combine_results(res):
    return _combine([res.results[c]["out_acc"] for c in range(N_CORES)],
                    _CACHE["last_possum"])
